# revision 2
# baseline (speedup 1.0000x reference)
"""Trainium2 Bass kernel for nn_CatEmbedder (gnn_message_passing).

Takes FULL inputs, shards batch B=32768 across 8 NeuronCores (4096 each),
replicates the embedding table + weights, runs an SPMD Bass kernel, and
concatenates the per-core outputs.

Per-core pipeline (32 blocks x 128 samples):
  1. indirect-DMA gather: emb[p, f*64:(f+1)*64] = table[idx[p,f]]  ([128,3200])
  2. PE transposes of [128,128] chunks (2 fields each) -> PSUM -> SBUF (et)
  3. squares (ACT/DVE split) into interleaved sq chunks
  4. PE seg-matmuls: field-sum + field-sum-of-squares  -> [64,256] PSUM
  5. PE u-matmuls: u_f = (S + PROBE*support_f)/c + ga_b  (bias via aug row)
  6. ACT relu-evict (bf16) -> PE accumulates sum_f relu(u_f)
  7. transposed MLPs for global/local branches, combine, transpose back, store
"""

import os
import sys
import numpy as np

sys.path.insert(0, "/opt/trn_rl_repo")

# ---- problem constants (hardcoded per the contract) ----
B, F, D, NCT = 32768, 50, 64, 100000
PROBE, ALPHA = 39.0, 0.5
NF = F + 1              # 51 fields
CD = NF + PROBE         # 90.0
NCORES = 8
BS = B // NCORES        # 4096 samples per core
BLK = 128
SUPER = 8               # idx/numf superblock (blocks per DMA)
NBLK_FULL = BS // BLK   # 32

USE_F32R = False        # fast fp32 matmul mode (walrus needs fp32r-typed producers)

_CACHE = {}


def _build(nblk=NBLK_FULL, reps=1):
    import concourse.bass as bass
    import concourse.mybir as mybir
    import concourse.tile as tile
    from concourse import bacc
    from contextlib import ExitStack

    f32 = mybir.dt.float32
    f32r = mybir.dt.float32r
    bf16 = mybir.dt.bfloat16
    i32 = mybir.dt.int32
    AL = mybir.AluOpType
    AF = mybir.ActivationFunctionType

    def r(ap):  # fp32 -> fp32r view for fast matmuls
        return ap.bitcast(f32r) if USE_F32R else ap

    nc = bacc.Bacc(None)

    # ---- DRAM parameters (order matters only for debug; bound by name) ----
    idx_d = nc.declare_dram_parameter("cat_idx", [BS, F], i32, isOutput=False)
    numf_d = nc.declare_dram_parameter("numf", [BS], f32, isOutput=False)
    table_d = nc.declare_dram_parameter("table", [NCT, D], f32, isOutput=False)
    ident_d = nc.declare_dram_parameter("ident128", [128, 128], f32, isOutput=False)
    segf_d = nc.declare_dram_parameter("seg_f", [128, D], f32, isOutput=False)
    segb_d = nc.declare_dram_parameter("seg_b", [128, D], bf16, isOutput=False)
    i64b_d = nc.declare_dram_parameter("i64_b", [D, D], bf16, isOutput=False)
    i64f_d = nc.declare_dram_parameter("i64_f", [D, D], f32, isOutput=False)
    waug_d = nc.declare_dram_parameter("waug", [D + 1, 128], f32, isOutput=False)
    gw2_d = nc.declare_dram_parameter("gw2", [128, 128], f32, isOutput=False)
    g0_d = nc.declare_dram_parameter("g0T", [D, D], f32, isOutput=False)
    g1_d = nc.declare_dram_parameter("g1aug", [D + 1, D], f32, isOutput=False)
    l0_d = nc.declare_dram_parameter("l0T", [D, D], f32, isOutput=False)
    l1_d = nc.declare_dram_parameter("l1aug", [D + 1, D], f32, isOutput=False)
    cols_d = nc.declare_dram_parameter("cols", [D, 4], f32, isOutput=False)
    ones_d = nc.declare_dram_parameter("ones164", [1, D], f32, isOutput=False)
    out_d = nc.declare_dram_parameter("out", [BS, D], f32, isOutput=True)

    GROUPS = [(0, 4), (4, 4), (8, 4), (12, 4), (16, 4), (20, 4), (24, 1)]
    NCHUNK = 25  # 25 chunks of 128 cols (2 fields each)

    with tile.TileContext(nc) as tc, ExitStack() as ctx:
        const = ctx.enter_context(tc.tile_pool(name="const", bufs=1))
        sb = ctx.enter_context(tc.tile_pool(name="sb", bufs=2))
        pst = ctx.enter_context(tc.tile_pool(name="pst", bufs=2, space="PSUM"))
        psu = ctx.enter_context(tc.tile_pool(name="psu", bufs=2, space="PSUM"))
        pseg = ctx.enter_context(tc.tile_pool(name="pseg", bufs=1, space="PSUM"))
        pracc = ctx.enter_context(tc.tile_pool(name="pracc", bufs=1, space="PSUM"))
        psm = ctx.enter_context(tc.tile_pool(name="psm", bufs=2, space="PSUM"))

        # ---- load constants once ----
        ident_t = const.tile([128, 128], f32)
        nc.sync.dma_start(ident_t[:], ident_d[:])
        segf_t = const.tile([128, D], f32)
        nc.sync.dma_start(segf_t[:], segf_d[:])
        segb_t = const.tile([128, D], bf16)
        nc.sync.dma_start(segb_t[:], segb_d[:])
        i64b_t = const.tile([D, D], bf16)
        nc.sync.dma_start(i64b_t[:], i64b_d[:])
        i64f_t = const.tile([D, D], f32)
        nc.sync.dma_start(i64f_t[:], i64f_d[:])
        waug_t = const.tile([D + 1, 128], f32)
        nc.sync.dma_start(waug_t[:], waug_d[:])
        gw2_t = const.tile([128, 128], f32)
        nc.sync.dma_start(gw2_t[:], gw2_d[:])
        g0_t = const.tile([D, D], f32)
        nc.sync.dma_start(g0_t[:], g0_d[:])
        g1_t = const.tile([D + 1, D], f32)
        nc.sync.dma_start(g1_t[:], g1_d[:])
        l0_t = const.tile([D, D], f32)
        nc.sync.dma_start(l0_t[:], l0_d[:])
        l1_t = const.tile([D + 1, D], f32)
        nc.sync.dma_start(l1_t[:], l1_d[:])
        cols_t = const.tile([D, 4], f32)
        nc.sync.dma_start(cols_t[:], cols_d[:])
        ones_t = const.tile([1, D], f32)
        nc.sync.dma_start(ones_t[:], ones_d[:])
        onesrow_t = const.tile([1, 128], f32)
        nc.vector.memset(onesrow_t[:], 1.0)

        numw_c = cols_t[:, 0:1]
        numb_c = cols_t[:, 1:2]
        gb0_c = cols_t[:, 2:3]
        lb0_c = cols_t[:, 3:4]

        idx_view = idx_d[:].rearrange("(s k p) f -> s p k f", p=BLK, k=SUPER)

        idx_t = None
        numf_t = None
        rep_cm = tc.For_i(0, reps, 1) if reps > 1 else None
        if rep_cm is not None:
            rep_cm.__enter__()
        for blk in range(nblk):
            s = blk % SUPER
            if s == 0:
                si = blk // SUPER
                idx_t = sb.tile([128, SUPER * F], i32, tag="idx")
                nc.sync.dma_start(
                    idx_t[:].rearrange("p (k f) -> p k f", k=SUPER), idx_view[si]
                )
                numf_t = sb.tile([1, SUPER * BLK], f32, tag="numf")
                nc.sync.dma_start(
                    numf_t[:], numf_d[None, si * SUPER * BLK:(si + 1) * SUPER * BLK]
                )

            # ---- 1. gather (one indirect DMA per field: HW consumes one
            # index per output partition) ----
            emb = sb.tile([128, F * D], f32, tag="emb")
            for f in range(F):
                nc.gpsimd.indirect_dma_start(
                    out=emb[:, f * D:(f + 1) * D],
                    out_offset=None,
                    in_=table_d[:, :],
                    in_offset=bass.IndirectOffsetOnAxis(
                        ap=idx_t[:, s * F + f:s * F + f + 1], axis=0
                    ),
                )

            # ---- numeric-field embedding (transposed): num_embT [64, 128] ----
            nrep = psm.tile([D, 128], f32, tag="small")
            nc.tensor.matmul(
                out=nrep[:], lhsT=ones_t[:],
                rhs=numf_t[:, s * BLK:(s + 1) * BLK],
                start=True, stop=True,
            )
            numembT = sb.tile([D, 128], f32, tag="numembT")
            nc.scalar.activation(
                out=numembT[:], in_=nrep[:], func=AF.Identity,
                bias=numb_c, scale=numw_c,
            )

            # ---- 2. transposes + evict; 3. squares ----
            # et layout: [128, 25*256] chunks [embT(128) | sq(128)]
            et = sb.tile([128, NCHUNK * 256], f32, tag="et")
            etv = et[:].rearrange("p (j c) -> p j c", c=256)
            for gi, (g0, gn) in enumerate(GROUPS):
                trp = pst.tile([128, 512], f32, tag="tr")
                for jj in range(gn):
                    j = g0 + jj
                    nc.tensor.matmul(
                        out=r(trp[:, jj * 128:(jj + 1) * 128]),
                        lhsT=r(emb[:, j * 128:(j + 1) * 128]),
                        rhs=r(ident_t[:]),
                        is_transpose=True, start=True, stop=True,
                    )
                src = trp[:, :gn * 128].rearrange("p (j c) -> p j c", c=128)
                nc.vector.tensor_copy(out=etv[:, g0:g0 + gn, 0:128], in_=src)
                if gi < 4:
                    # square on ACT straight from PSUM
                    nc.scalar.activation(
                        out=etv[:, g0:g0 + gn, 128:256], in_=src, func=AF.Square,
                    )
                else:
                    # square on DVE from SBUF (after evict)
                    nc.vector.tensor_tensor(
                        out=etv[:, g0:g0 + gn, 128:256],
                        in0=etv[:, g0:g0 + gn, 0:128],
                        in1=etv[:, g0:g0 + gn, 0:128],
                        op=AL.mult,
                    )

            # ---- 4. seg-matmuls: [sumT | sumsqT] accumulate in [64, 256] ----
            seg = pseg.tile([D, 256], f32, tag="seg")
            for j in range(NCHUNK):
                nc.tensor.matmul(
                    out=seg[:],
                    lhsT=r(segf_t[:]),
                    rhs=r(et[:, j * 256:(j + 1) * 256]),
                    start=(j == 0), stop=(j == NCHUNK - 1),
                    skip_group_check=True,
                )

            # ---- summedT (+aug ones row) ----
            saug = sb.tile([D + 1, 128], f32, tag="saug")
            nc.vector.tensor_tensor(
                out=saug[0:D, :], in0=seg[:, 0:128], in1=numembT[:], op=AL.add,
            )
            nc.vector.tensor_copy(out=saug[D:D + 1, :], in_=onesrow_t[:])

            # ---- 5. u-matmuls ----
            # bias: one N=512 matmul with rhs = summedT_aug repeated 4x
            saug_rep = (
                saug[:].rearrange("p (o n) -> p o n", o=1)
                .to_broadcast([D + 1, 4, 128])
            )
            r_buf = sb.tile([128, NCHUNK * 128], bf16, tag="rbuf")
            for gi, (g0, gn) in enumerate(GROUPS):
                up = psu.tile([128, 512], f32, tag="u")
                if gn == 4:
                    nc.tensor.matmul(
                        out=up[:], lhsT=r(waug_t[:]), rhs=r(saug_rep),
                        start=True, stop=False, skip_group_check=True,
                    )
                    for pp in range(2):  # support pairs N=256
                        rhs = et[:].rearrange("p (j c) -> p j c", c=256)[
                            :, g0 + 2 * pp:g0 + 2 * pp + 2, 0:128
                        ]
                        nc.tensor.matmul(
                            out=up[:, pp * 256:(pp + 1) * 256],
                            lhsT=r(gw2_t[:]), rhs=r(rhs),
                            start=False, stop=True, skip_group_check=True,
                        )
                else:
                    nc.tensor.matmul(
                        out=up[:, 0:128], lhsT=r(waug_t[:]), rhs=r(saug[:]),
                        start=True, stop=False, skip_group_check=True,
                    )
                    nc.tensor.matmul(
                        out=up[:, 0:128], lhsT=r(gw2_t[:]),
                        rhs=r(et[:, g0 * 256:g0 * 256 + 128]),
                        start=False, stop=True, skip_group_check=True,
                    )
                # ---- 6. relu-evict to bf16 ----
                nc.scalar.activation(
                    out=r_buf[:, g0 * 128:(g0 + gn) * 128],
                    in_=up[:, :gn * 128], func=AF.Relu,
                )

            # num field u + relu
            unum = psm.tile([D, 128], f32, tag="small")
            nc.tensor.matmul(
                out=unum[:], lhsT=r(waug_t[:, 0:D]), rhs=r(saug[:]),
                start=True, stop=False, skip_group_check=True,
            )
            nc.tensor.matmul(
                out=unum[:], lhsT=r(gw2_t[0:D, 0:D]), rhs=r(numembT[:]),
                start=False, stop=True, skip_group_check=True,
            )
            rnum = sb.tile([D, 128], bf16, tag="rnum")
            nc.scalar.activation(out=rnum[:], in_=unum[:], func=AF.Relu)

            # ---- racc: g_preT = sum_f relu(u_f) ----
            gpre = pracc.tile([D, 128], f32, tag="gpre")
            for j in range(NCHUNK):
                nc.tensor.matmul(
                    out=gpre[:], lhsT=segb_t[:], rhs=r_buf[:, j * 128:(j + 1) * 128],
                    start=(j == 0), stop=False, skip_group_check=True,
                )
            nc.tensor.matmul(
                out=gpre[:], lhsT=i64b_t[:], rhs=rnum[:],
                start=False, stop=True, skip_group_check=True,
            )
            gpreT = sb.tile([D, 128], f32, tag="gpreT")
            nc.scalar.copy(out=gpreT[:], in_=gpre[:])

            # ---- local branch: lT = summedT^2 - sumsqT ----
            lT = sb.tile([D, 128], f32, tag="lT")
            nc.vector.tensor_tensor(
                out=lT[:], in0=saug[0:D, :], in1=saug[0:D, :], op=AL.mult,
            )
            nc.vector.tensor_tensor(
                out=lT[:], in0=lT[:], in1=seg[:, 128:256], op=AL.subtract,
            )

            # ---- MLPs (transposed) ----
            h1p = psm.tile([D, 128], f32, tag="small")
            nc.tensor.matmul(out=h1p[:], lhsT=g0_t[:], rhs=gpreT[:],
                             start=True, stop=True)
            h1aug = sb.tile([D + 1, 128], f32, tag="h1aug")
            nc.scalar.activation(out=h1aug[0:D, :], in_=h1p[:], func=AF.Relu,
                                 bias=gb0_c)
            nc.vector.tensor_copy(out=h1aug[D:D + 1, :], in_=onesrow_t[:])

            l1p = psm.tile([D, 128], f32, tag="small")
            nc.tensor.matmul(out=l1p[:], lhsT=l0_t[:], rhs=lT[:],
                             start=True, stop=True)
            l1aug = sb.tile([D + 1, 128], f32, tag="l1aug")
            nc.scalar.activation(out=l1aug[0:D, :], in_=l1p[:], func=AF.Relu,
                                 bias=lb0_c)
            nc.vector.tensor_copy(out=l1aug[D:D + 1, :], in_=onesrow_t[:])

            outp = psm.tile([D, 128], f32, tag="small")
            nc.tensor.matmul(out=outp[:], lhsT=g1_t[:], rhs=h1aug[:],
                             start=True, stop=False, skip_group_check=True)
            nc.tensor.matmul(out=outp[:], lhsT=l1_t[:], rhs=l1aug[:],
                             start=False, stop=True, skip_group_check=True)
            outT = sb.tile([D, 128], f32, tag="outT")
            nc.scalar.copy(out=outT[:], in_=outp[:])

            # ---- transpose back to [128, 64] and store ----
            finp = psm.tile([128, D], f32, tag="small")
            nc.tensor.matmul(out=finp[:], lhsT=outT[:], rhs=i64f_t[:],
                             is_transpose=True, start=True, stop=True)
            orow = sb.tile([128, D], f32, tag="orow")
            nc.vector.tensor_copy(out=orow[:], in_=finp[:])
            nc.sync.dma_start(out_d[blk * BLK:(blk + 1) * BLK, :], orow[:])

        if rep_cm is not None:
            rep_cm.__exit__(None, None, None)

    return nc


def _make_consts(embed_table, num_W, num_b, ga_W, ga_b, gW, gb, lW, lb):
    """Host-side constant prep. Returns dict of name -> np.ndarray."""
    f = np.float32
    ga_W = ga_W.astype(f)
    ident128 = np.eye(128, dtype=f)
    i64 = np.eye(D, dtype=f)
    seg = np.vstack([i64, i64]).astype(f)           # [128, 64]
    waug = np.zeros((D + 1, 128), f)                # bias matmul lhsT
    waug[:D, :D] = ga_W / CD
    waug[:D, D:] = ga_W / CD
    waug[D, :D] = ga_b
    waug[D, D:] = ga_b
    gw2 = np.zeros((128, 128), f)                   # blockdiag support lhsT
    gw2[:D, :D] = ga_W * (PROBE / CD)
    gw2[D:, D:] = ga_W * (PROBE / CD)
    g0T = (gW[0].astype(f) / NF).T.copy()           # fold 1/51 mean
    g1aug = np.zeros((D + 1, D), f)
    g1aug[:D] = ALPHA * gW[1].astype(f).T
    g1aug[D] = ALPHA * gb[1].astype(f)
    l0T = (0.5 * lW[0].astype(f)).T.copy()          # fold FM 0.5
    l1aug = np.zeros((D + 1, D), f)
    l1aug[:D] = (1.0 - ALPHA) * lW[1].astype(f).T
    l1aug[D] = (1.0 - ALPHA) * lb[1].astype(f)
    cols = np.stack(
        [num_W[:, 0].astype(f), num_b.astype(f), gb[0].astype(f), lb[0].astype(f)],
        axis=1,
    ).copy()                                        # [64, 4]
    return {
        "table": np.ascontiguousarray(embed_table.astype(f)),
        "ident128": ident128,
        "seg_f": seg,
        "seg_b": seg,          # cast to bf16 at map build
        "i64_b": i64,          # cast to bf16 at map build
        "i64_f": i64,
        "waug": waug,
        "gw2": gw2,
        "g0T": g0T,
        "g1aug": g1aug,
        "l0T": l0T,
        "l1aug": l1aug,
        "cols": cols,
        "ones164": np.ones((1, D), f),
    }


def _get_nc():
    if "nc" not in _CACHE:
        print("[kernel] building bass module...", flush=True)
        nc = _build()
        print("[kernel] finalizing...", flush=True)
        nc.finalize()
        _CACHE["nc"] = nc
        print("[kernel] build done", flush=True)
    return _CACHE["nc"]


def _make_in_maps(inputs):
    """inputs: dict with the reference's setup_inputs() keys."""
    import ml_dtypes

    consts = _make_consts(
        inputs["embed_table"], inputs["num_W"], inputs["num_b"],
        inputs["ga_W"], inputs["ga_b"], inputs["gW"], inputs["gb"],
        inputs["lW"], inputs["lb"],
    )
    bf = ml_dtypes.bfloat16
    cmap = {
        k: (v.astype(bf) if k in ("seg_b", "i64_b") else v)
        for k, v in consts.items()
    }

    idx32 = np.ascontiguousarray(np.asarray(inputs["cat_indices"]).astype(np.int32))
    numf = np.ascontiguousarray(
        np.asarray(inputs["num_features"]).astype(np.float32))

    in_maps = []
    for c in range(NCORES):
        m = dict(cmap)
        m["cat_idx"] = np.ascontiguousarray(idx32[c * BS:(c + 1) * BS])
        m["numf"] = np.ascontiguousarray(numf[c * BS:(c + 1) * BS])
        in_maps.append(m)
    return in_maps


def kernel(cat_indices, num_features, embed_table, num_W, num_b,
           ga_W, ga_b, gW, gb, lW, lb):
    from concourse.bass_utils import run_bass_kernel_spmd

    nc = _get_nc()
    in_maps = _make_in_maps(dict(
        cat_indices=cat_indices, num_features=num_features,
        embed_table=embed_table, num_W=num_W, num_b=num_b,
        ga_W=ga_W, ga_b=ga_b, gW=gW, gb=gb, lW=lW, lb=lb,
    ))

    print("[kernel] launching spmd run...", flush=True)
    res = run_bass_kernel_spmd(nc, in_maps, list(range(NCORES)))
    print("[kernel] run complete", flush=True)
    outs = [res.results[c]["out"] for c in range(NCORES)]
    return np.concatenate(outs, axis=0).astype(np.float32)

